# revision 57
# baseline (speedup 1.0000x reference)
"""Multi-head attention (B=4, S=2048, D=1024, H=16, DK=64) on 8 TRN2 cores.

Sharding: core c = (b, g) with b = c//2 in 0..3 (data parallel on batch) and
g = c%2 (tensor parallel on heads: 8 heads / 512 d' columns per group).
Each core computes a partial output projection; the host sums the two
partials per batch (the "all-reduce" of the sharding hint, done host-side)
and adds bo.

Per-core device algorithm (all matmul inputs bf16, fp32 PSUM accumulation):
  QT[d',q] = Wq_g^T Xq^T (+bq)          via lhsT=Wq tiles, rhs=XqT tiles;
                                        bias folded into the PSUM->SBUF
                                        eviction as a per-partition
                                        tensor_scalar_add (no bias matmul)
  KT[d',k] = same with Wk
  Vn[k,d'] = Xv Wv_g + 1 bv^T           natural layout, plus a ones column
                                        per head -> V_aug [k, 65] per head
  per (q-chunk, head):
    ST[k,q]   = KT_h^T-style scores via lhsT=KT slice, rhs=QT slice
    P = exp(ST/8)                       one ACT pass per 2 k-tiles (PSUM->SBUF)
    AT_aug    = sum_k V_aug^T P         -> [65, q]: row 0 = softmax
                                           denominators, rows 1..64 = V^T P
    r = 1/AT_aug[0]                     DVE reciprocal_approx_fast (fp32,
                                        partition 0 -- the custom op only
                                        works at partition offset 0)
    bc[128,q] = broadcast of r          two K=1 PE matmuls (lhsT=ones row,
                                        rhs=r row) -> PSUM, no DRAM bounce
    ATn = AT_pair * bc                  one in-place DVE tensor_mul per pair,
                                        drip-fed into the next pair's k-loop
                                        so the latency chain never stalls PE
  out[q,e] += sum_h ATn_h^T Wo_h        partial output projection, stored
                                        bf16 (host accumulates in fp32)
"""

import os
import sys
import time
import types

sys.path.insert(0, "/opt/trn_rl_repo")

import numpy as np
import ml_dtypes

# ---------------------------------------------------------------------------
# axon NTFF profile hook (missing from this image's antenv stub); harmless
# when tracing is disabled.
# ---------------------------------------------------------------------------
def _install_axon_hooks():
    import antenv

    if "antenv.axon_hooks" in sys.modules:
        return
    hooks = types.ModuleType("antenv.axon_hooks")
    hooks._hook = None
    hooks.set_axon_ntff_profile_hook = lambda h: setattr(hooks, "_hook", h)
    hooks.get_axon_ntff_profile_hook = lambda: hooks._hook
    sys.modules["antenv.axon_hooks"] = hooks
    antenv.axon_hooks = hooks
    try:
        from trn_agent_boot.trn_boot import _ntff_profile_via_ctypes

        hooks.set_axon_ntff_profile_hook(
            _ntff_profile_via_ctypes("/opt/axon/libaxon_pjrt.so")
        )
    except Exception:
        pass


_install_axon_hooks()

import concourse.bacc as bacc
import concourse.bass as bass
import concourse.tile as tile
from concourse import mybir
from concourse import bass_utils
from concourse.bass_utils import run_bass_kernel_spmd

# The trace path uploads artifacts to a network bucket; keep it local.
bass_utils.upload_artifacts = lambda tmpdir: tmpdir

BF16 = mybir.dt.bfloat16
F16 = mybir.dt.float16
F32 = mybir.dt.float32

# Problem dims (hardcoded per spec)
B, S, D = 4, 2048, 1024
H, DK = 16, 64
N_CORES = 8
HC = H // N_CORES * B  # heads per core = 8  (16 heads / 2 groups)
DPC = HC * DK  # d' columns per core = 512

LAST_EXEC_TIME_NS = None

# kk index (within the next pair's k-loop) at which the previous pair's
# normalization broadcast matmuls / multiply are emitted. Late enough that
# the DVE reciprocals have completed (no PE stall), early enough that the
# out-projection fillers reading the normalized atn come after.
NORM_MM_KK = 4
NORM_MUL_KK = 5


def build_program(s=S, dm=D, hc=HC, e=D):
    """Build the per-core Bass program. All dims in units of elements.

    s: sequence length (multiple of 512), dm: model dim (multiple of 128),
    hc: heads per core (even), e: output model dim (multiple of 512).
    """
    dk = DK
    dpc = hc * dk  # d' per core
    pairs = hc // 2
    dt_n = dm // 128  # d-tiles (contraction tiles for projections)
    st_n = s // 128  # s-tiles = k-tiles
    qc_n = s // 512  # q-chunks
    ec_n = e // 512  # out-proj column chunks

    nc = bacc.Bacc("TRN2", target_bir_lowering=False, debug=False,
                   num_devices=N_CORES)

    xqT = nc.dram_tensor("xqT", [dm, s], BF16, kind="ExternalInput")
    xkT = nc.dram_tensor("xkT", [dm, s], BF16, kind="ExternalInput")
    xvT = nc.dram_tensor("xvT", [dm, s], BF16, kind="ExternalInput")
    wq = nc.dram_tensor("wq", [dm, dpc], BF16, kind="ExternalInput")
    wk = nc.dram_tensor("wk", [dm, dpc], BF16, kind="ExternalInput")
    wv = nc.dram_tensor("wv", [dm, dpc], BF16, kind="ExternalInput")
    wo = nc.dram_tensor("wo", [dpc, e], BF16, kind="ExternalInput")
    bq = nc.dram_tensor("bq", [dpc], F32, kind="ExternalInput")
    bk = nc.dram_tensor("bk", [dpc], F32, kind="ExternalInput")
    bv = nc.dram_tensor("bv", [dpc], BF16, kind="ExternalInput")
    out = nc.dram_tensor("out", [s, e], BF16, kind="ExternalOutput")

    with tile.TileContext(nc) as tc:
        with (
            tc.tile_pool(name="singles", bufs=1) as singles,
            tc.tile_pool(name="xin", bufs=2) as xin,
            tc.tile_pool(name="expst", bufs=6) as expst_pool,
            tc.tile_pool(name="atn", bufs=hc) as atn_pool,
            tc.tile_pool(name="small", bufs=3) as small,
            tc.tile_pool(name="outsb", bufs=3) as outsb_pool,
            tc.tile_pool(name="ps_sc", bufs=2, space="PSUM") as ps_sc,
            tc.tile_pool(name="ps_at", bufs=4, space="PSUM") as ps_at,
        ):
            # ---- persistent SBUF tensors ----
            qt_sb = singles.tile([128, pairs, s], BF16, tag="qt")
            kt_sb = singles.tile([128, pairs, s], BF16, tag="kt")
            vn_sb = singles.tile([128, st_n, hc, dk + 1], BF16, tag="vn")
            wq_sb = singles.tile([128, dt_n, dpc], BF16, tag="wq")
            wk_sb = singles.tile([128, dt_n, dpc], BF16, tag="wk")
            wv_sb = singles.tile([128, dt_n, dpc], BF16, tag="wv")
            wo_sb = singles.tile([128, pairs, e], BF16, tag="wo")
            bqT_sb = singles.tile([128, pairs], F32, tag="bqT")
            bkT_sb = singles.tile([128, pairs], F32, tag="bkT")
            bv_sb = singles.tile([1, dpc], BF16, tag="bv")
            ones_sb = singles.tile([128, 512], BF16, tag="ones")
            ones16_sb = singles.tile([1, 64], F16, tag="ones16")

            # x staging uses a column-block-major layout [p, block, t, cols]
            # so each DMA piece is one contiguous block and consumers only
            # wait for the blocks they read.
            xb_v = max(1, s // 256)   # V reads 128-wide slices
            xb_p = max(1, s // 512)   # projections read 512-wide slices

            # Input loads alternate between the (otherwise idle) GpSimd and
            # Scalar DMA queues: two queues double the early descriptor
            # throughput and nothing serializes behind the Sync queue.
            in_q = [0]

            def in_dma(out, in_):
                eng = nc.gpsimd if in_q[0] % 2 == 0 else nc.scalar
                in_q[0] += 1
                eng.dma_start(out=out, in_=in_)

            def load_x_blocked(xdram, nblk):
                x_sb = xin.tile([128, nblk, dt_n, s // nblk], BF16, tag="x")
                src = xdram.ap().rearrange("(t p) n -> p t n", p=128)
                for j in range(nblk):
                    jsl = slice(j * s // nblk, (j + 1) * s // nblk)
                    nc.gpsimd.dma_start(
                        out=x_sb[:, j, :, :], in_=src[:, :, jsl])
                return x_sb

            def xslice(x_sb, t, lo, width):
                nblk = x_sb.shape[1]
                bw = s // nblk
                j, off = lo // bw, lo % bw
                assert off + width <= bw
                return x_sb[:, j, t, off : off + width]

            # The first V-projection chain needs wv[dt] + xv block 0 only:
            # issue those as per-dt pieces, interleaved, so the first matmul
            # fires as soon as the first two small DMAs land instead of
            # waiting for two monolithic 0.5-1MB transfers.
            wv_src = wv.ap().rearrange("(t p) n -> p t n", p=128)
            xv_src = xvT.ap().rearrange("(t p) n -> p t n", p=128)
            bw_v = s // xb_v
            xv_sb = xin.tile([128, xb_v, dt_n, bw_v], BF16, tag="x")
            for t in range(dt_n):
                in_dma(wv_sb[:, t, :], wv_src[:, t, :])
                in_dma(xv_sb[:, 0, t, :], xv_src[:, t, 0:bw_v])
            nc.gpsimd.dma_start(
                out=bv_sb, in_=bv.ap().rearrange("(o n) -> o n", o=1))
            for j in range(1, xb_v):
                jsl = slice(j * bw_v, (j + 1) * bw_v)
                nc.gpsimd.dma_start(
                    out=xv_sb[:, j, :, :], in_=xv_src[:, :, jsl])
            nc.gpsimd.dma_start(
                out=wk_sb, in_=wk.ap().rearrange("(t p) n -> p t n", p=128))
            nc.gpsimd.dma_start(
                out=bkT_sb, in_=bk.ap().rearrange("(a p) -> p a", p=128))
            xk_sb = load_x_blocked(xkT, xb_p)
            nc.gpsimd.dma_start(
                out=wq_sb, in_=wq.ap().rearrange("(t p) n -> p t n", p=128))
            nc.gpsimd.dma_start(
                out=bqT_sb, in_=bq.ap().rearrange("(a p) -> p a", p=128))
            nc.gpsimd.dma_start(
                out=wo_sb, in_=wo.ap().rearrange("(a p) e -> p a e", p=128))
            nc.vector.memset(ones_sb, 1.0)
            nc.vector.memset(ones16_sb, 1.0)
            # ones column of every V_aug head block, at POSITION 0 so the
            # softmax denominator lands in PSUM partition 0 (where the fast
            # custom-DVE reciprocal works; it misbehaves at other offsets)
            nc.vector.memset(vn_sb[:, :, :, 0:1], 1.0)

            # Warm-up exp ACT right away: forces the ~2.7us ACT_TABLE_LOAD to
            # overlap the projection matmuls instead of stalling the PE >3.4us
            # at the start of attention (which re-throttles HAM to 1.2 GHz
            # for the whole attention phase).
            warm_sb = singles.tile([128, 32], F32, tag="warm")
            nc.scalar.activation(
                warm_sb, ones_sb[:, 0:32], mybir.ActivationFunctionType.Exp)

            # ---- helper closures ----
            def proj_qk_pair(w_sb, bT_sb, x_sb, dst, p, qc):
                """One [d' 128, q 512] projection chain for a head pair."""
                ps = ps_at.tile([128, 512], F32, tag="ps")
                for t in range(dt_n):
                    nc.tensor.matmul(
                        ps,
                        w_sb[:, t, p * 128 : (p + 1) * 128],
                        xslice(x_sb, t, qc * 512, 512),
                        start=(t == 0),
                        stop=(t == dt_n - 1),
                    )
                nc.vector.tensor_scalar_add(
                    dst[:, p, qc * 512 : (qc + 1) * 512], ps, bT_sb[:, p : p + 1])

            # ---- stage A upfront ----
            # V first (its DMAs were issued first); most of K/Q projection is
            # deferred into B(0) as PE filler.
            vw = min(512, dpc)
            for st in range(st_n):
                for nchunk in range(dpc // vw):
                    nsl = slice(nchunk * vw, (nchunk + 1) * vw)
                    ps = ps_at.tile([128, vw], F32, tag="ps")
                    for t in range(dt_n):
                        nc.tensor.matmul(
                            ps,
                            xslice(xv_sb, t, st * 128, 128),
                            wv_sb[:, t, nsl],
                            start=(t == 0),
                            stop=False,
                        )
                    nc.tensor.matmul(
                        ps,
                        ones_sb[0:1, 0:128],
                        bv_sb[0:1, nsl],
                        start=False,
                        stop=True,
                    )
                    nc.vector.tensor_copy(
                        vn_sb[
                            :, st,
                            nchunk * (vw // dk) : (nchunk + 1) * (vw // dk),
                            1 : dk + 1,
                        ],
                        ps.rearrange("p (h d) -> p h d", d=dk),
                    )

            # xq reuses xv's slot (frees after the last V matmul)
            xq_sb = load_x_blocked(xqT, xb_p)
            # KT pairs 0..2 upfront (pair 2 minus its last chain); pair 3 +
            # the rest + QT0 pairs 1..3 fill B(0)
            for p in range(min(3, pairs)):
                for qcc in range(qc_n):
                    if p == 2 and qcc == qc_n - 1:
                        continue  # deferred into B(0), deadline (0, 2)
                    proj_qk_pair(wk_sb, bkT_sb, xk_sb, kt_sb, p, qcc)
            proj_qk_pair(wq_sb, bqT_sb, xq_sb, qt_sb, 0, 0)

            # ---- filler generators (drip-fed PE work) ----
            def proj_qk_gen(w_sb, bT_sb, x_sb, dst, p, qcc):
                """Projection chain yielding after each matmul."""
                ps = ps_at.tile([128, 512], F32, tag="ps")
                for t in range(dt_n):
                    nc.tensor.matmul(
                        ps,
                        w_sb[:, t, p * 128 : (p + 1) * 128],
                        xslice(x_sb, t, qcc * 512, 512),
                        start=(t == 0),
                        stop=(t == dt_n - 1),
                    )
                    yield
                nc.vector.tensor_scalar_add(
                    dst[:, p, qcc * 512 : (qcc + 1) * 512], ps,
                    bT_sb[:, p : p + 1])
                yield

            def outproj_gen(atn_q, qcc, qt_i, ecc):
                """Out-projection sequence (pair-packed, K=128 per matmul)."""
                esl = slice(ecc * 512, (ecc + 1) * 512)
                q0 = qcc * 4 + qt_i
                o_ps = ps_at.tile([128, 512], F32, tag="ps")
                for p in range(pairs):
                    nc.tensor.matmul(
                        o_ps,
                        atn_q[p][:, qt_i * 128 : (qt_i + 1) * 128],
                        wo_sb[:, p, esl],
                        start=(p == 0),
                        stop=(p == pairs - 1),
                    )
                    yield
                o_sb = outsb_pool.tile([128, 512], BF16, tag="o")
                nc.vector.tensor_copy(o_sb, o_ps)
                nc.sync.dma_start(
                    out=out.ap()[q0 * 128 : (q0 + 1) * 128, esl], in_=o_sb)
                yield

            class FillerQueue:
                def __init__(self):
                    self.tasks = []  # (gen, deadline_pr or None)

                def add(self, gen, deadline=None):
                    self.tasks.append((gen, deadline))

                def pump(self, n):
                    while n > 0 and self.tasks:
                        try:
                            next(self.tasks[0][0])
                            n -= 1
                        except StopIteration:
                            self.tasks.pop(0)

                def fence(self, pr):
                    # complete every task whose deadline is <= pr (FIFO order
                    # matches deadline order)
                    while self.tasks and any(
                        dl is not None and dl <= pr for _, dl in self.tasks
                    ):
                        self.pump(1000)

                def drain(self):
                    while self.tasks:
                        self.pump(1000)

            # ---- softmax normalization (drip-fed across pair boundaries) --
            # After a pair's AV accumulation, two DVE reciprocal_approx_fast
            # ops turn the PSUM denominator rows into rates; during the NEXT
            # pair's k-loop two K=1 matmuls broadcast them across partitions
            # into PSUM and one in-place DVE multiply normalizes the pair's
            # atn tile. No DRAM bounce, no gpsimd, no per-head tail chain.
            pending_norm = []  # [{rec, atn, bc}]

            def norm_mms(item):
                bc = ps_at.tile([128, 512], F32, tag="ps")
                nc.tensor.matmul(
                    bc[0:64, :], ones16_sb,
                    item["rec"][0:1, 0:512], start=True, stop=True)
                nc.tensor.matmul(
                    bc[64:128, :], ones16_sb,
                    item["rec"][0:1, 512:1024], start=True, stop=True)
                item["bc"] = bc

            def norm_mul(item):
                nc.vector.tensor_mul(item["atn"], item["atn"], item["bc"])

            # ---- stages B+C interleaved over q-chunks ----
            # B(qc) processes HEAD PAIRS: the two heads' score matmuls run
            # concurrently in different PE row groups (K=64 each); one
            # [128, 2048] exp ACT covers 2 k-tiles x both heads. Filler
            # matmuls (C(qc-1) out-projections, QT(qc+1) projections) are
            # drip-fed between groups to keep the PE dense for HAM.
            def emit_sc_exp(pr_, qc_, kk_):
                """Score matmuls + exp for one (pair, q-chunk, k-tile) unit."""
                qsl_ = slice(qc_ * 512, (qc_ + 1) * 512)
                ksl_ = slice(kk_ * 128, (kk_ + 1) * 128)
                sc_ps = ps_sc.tile([128, 1024], F32, tag="sc")
                nc.tensor.matmul(
                    sc_ps[:, 0:512],
                    kt_sb[0:64, pr_, ksl_],
                    qt_sb[0:64, pr_, qsl_],
                    start=True,
                    stop=True,
                )
                nc.tensor.matmul(
                    sc_ps[:, 512:1024],
                    kt_sb[64:128, pr_, ksl_],
                    qt_sb[64:128, pr_, qsl_],
                    start=True,
                    stop=True,
                )
                exp_sb = expst_pool.tile([128, 1024], BF16, tag="e")
                nc.scalar.activation(
                    exp_sb, sc_ps,
                    mybir.ActivationFunctionType.Exp,
                    scale=1.0 / np.sqrt(dk),
                )
                return exp_sb

            # ONE filler queue persists across q-chunks: leftovers from a
            # chunk spread into the next chunk's pump slots instead of
            # draining as a serial burst at the boundary (which starved the
            # exp stream for ~10us). Deadlines are (qc, pair) tuples.
            fill = FillerQueue()
            carry_pipe = []
            n_tail_defer = 4  # qc2 out-proj seqs kept as tail norm cover
            prev_atn = None
            for qc in range(qc_n):
                qsl = slice(qc * 512, (qc + 1) * 512)
                last = qc == qc_n - 1
                atn_q = []

                if qc == 0:
                    if pairs >= 3:
                        fill.add(
                            proj_qk_gen(
                                wk_sb, bkT_sb, xk_sb, kt_sb, 2, qc_n - 1),
                            deadline=(0, 2),
                        )
                    for pp in range(1, pairs):
                        fill.add(
                            proj_qk_gen(wq_sb, bqT_sb, xq_sb, qt_sb, pp, 0),
                            deadline=(0, pp),
                        )
                        if pairs == 4 and pp < 4:
                            fill.add(
                                proj_qk_gen(
                                    wk_sb, bkT_sb, xk_sb, kt_sb, 3, pp - 1),
                                deadline=(0, 3),
                            )
                    if pairs == 4:
                        fill.add(
                            proj_qk_gen(wk_sb, bkT_sb, xk_sb, kt_sb, 3, 3),
                            deadline=(0, 3),
                        )
                # QT projections first: they are never gated, while the
                # out-projections of qc-1 wait on its normalization chain --
                # a stalled filler matmul blocks the whole PE FIFO.
                if qc + 1 < qc_n:
                    for pp in range(pairs):
                        fill.add(
                            proj_qk_gen(
                                wq_sb, bqT_sb, xq_sb, qt_sb, pp, qc + 1),
                            deadline=(qc + 1, pp),
                        )
                if prev_atn is not None:
                    n_seq = 4 * ec_n - (n_tail_defer if last else 0)
                    for sq in range(n_seq):
                        fill.add(
                            outproj_gen(
                                prev_atn, qc - 1, sq // ec_n, sq % ec_n),
                            # backstop only: atn(qc-1) tiles must free
                            # before B(qc+1) reuses their pool slots; pair 1
                            # (not 0) so leftovers drain via pair-0 pumps
                            # instead of bursting at the boundary fence
                            deadline=(qc + 1, 1),
                        )

                for pr in range(pairs):
                    fill.fence((qc, pr))
                    at_A = ps_at.tile([65, 512], F32, tag="ps")
                    at_B = ps_at.tile([65, 512], F32, tag="ps")
                    # AT matmuls lag the score/exp stream by TWO units, so
                    # an exp-gated AT-pair never sits between consecutive
                    # score units in the PE queue. The first two units of
                    # this pair may have been hoisted across the boundary.
                    pend = carry_pipe
                    carry_pipe = []

                    def emit_at(kk, e_sb):
                        nc.tensor.matmul(
                            at_A,
                            vn_sb[:, kk, 2 * pr, :],
                            e_sb[:, 0:512],
                            start=(kk == 0),
                            stop=(kk == st_n - 1),
                        )
                        nc.tensor.matmul(
                            at_B,
                            vn_sb[:, kk, 2 * pr + 1, :],
                            e_sb[:, 512:1024],
                            start=(kk == 0),
                            stop=(kk == st_n - 1),
                        )

                    if pr + 1 < pairs:
                        nxt = (qc, pr + 1)
                    elif qc + 1 < qc_n:
                        nxt = (qc + 1, 0)
                    else:
                        nxt = None

                    for kk in range(len(pend), st_n):
                        exp_sb = emit_sc_exp(pr, qc, kk)
                        # fillers go BEFORE the exp-gated AT matmuls of the
                        # previous units: they execute inside the exp-wait
                        # window instead of delaying the next score matmuls
                        if last and pr == 0 and kk <= NORM_MUL_KK:
                            pass  # out-projection fillers of qc-1 must come
                                  # after its pair-3 normalization below
                        elif qc == 0 and pr < pairs - 1:
                            # B(0) must absorb the QT(0)/KT(3) chains before
                            # their fences; 3/2 per unit rides the ACT slack
                            # instead of bursting ~15 chains at the pair-3
                            # fence (which starved the exp stream ~10us)
                            fill.pump(2 if kk % 2 == 0 else 1)
                        else:
                            fill.pump(2 if last or pr == pairs - 1 else 1)
                        pend.append((kk, exp_sb))
                        if len(pend) > 2:
                            emit_at(*pend.pop(0))
                        if kk == NORM_MM_KK and pending_norm:
                            norm_mms(pending_norm[0])
                        if kk == NORM_MUL_KK and pending_norm:
                            norm_mul(pending_norm.pop(0))
                    if nxt is not None:
                        # hoist the next pair's first two score units ahead
                        # of this pair's trailing (exp-gated) AT matmuls so
                        # the exp stream crosses the boundary seamlessly
                        fill.fence(nxt)
                        carry_pipe = [
                            (0, emit_sc_exp(nxt[1], nxt[0], 0)),
                            (1, emit_sc_exp(nxt[1], nxt[0], 1)),
                        ]
                    for item in pend:
                        emit_at(*item)

                    # pair tile: AT lives at PSUM rows 1:65 (denominator at
                    # row 0), so both heads move into place via SBUF->SBUF
                    # DMA partition shifts after a same-partition DVE cast
                    atn_pair = atn_pool.tile([128, 512], BF16, tag="atn")
                    btmp_a = small.tile([65, 512], BF16, tag="btmp")
                    rec = small.tile([1, 1024], F32, tag="rec")
                    nc.vector.tensor_copy(btmp_a, at_A[0:65, :])
                    nc.vector.reciprocal_approx_fast(
                        out=rec[0:1, 0:512], in_=at_A[0:1, :])
                    nc.gpsimd.dma_start(
                        out=atn_pair[0:64, :], in_=btmp_a[1:65, :])
                    btmp_b = small.tile([65, 512], BF16, tag="btmp")
                    nc.vector.tensor_copy(btmp_b, at_B[0:65, :])
                    nc.vector.reciprocal_approx_fast(
                        out=rec[0:1, 512:1024], in_=at_B[0:1, :])
                    nc.gpsimd.dma_start(
                        out=atn_pair[64:128, :], in_=btmp_b[1:65, :])
                    # cast to f16 so the broadcast matmul runs at full rate
                    rec16 = small.tile([1, 1024], F16, tag="rec16")
                    nc.vector.tensor_copy(rec16, rec)
                    pending_norm.append({"rec": rec16, "atn": atn_pair})
                    atn_q.append(atn_pair)

                prev_prev_atn, prev_atn = prev_atn, atn_q

            # ---- tail ----
            # The deferred qc2 sequences + partial qc3 sequences cover the
            # final pair's normalization chain; PSUM evictions go via the
            # (now idle) ACT engine and stores via the cheap gpsimd queue.
            tail_n = [0]

            def tail_evict(o_ps, qcc, qt_i, esl):
                # alternate the eviction copy and the store DMA across two
                # idle engines each, so neither serializes the tail
                q0 = qcc * 4 + qt_i
                o_sb = outsb_pool.tile([128, 512], BF16, tag="o")
                if tail_n[0] % 2 == 0:
                    nc.scalar.activation(
                        o_sb, o_ps, mybir.ActivationFunctionType.Copy)
                    nc.gpsimd.dma_start(
                        out=out.ap()[q0 * 128 : (q0 + 1) * 128, esl], in_=o_sb)
                else:
                    nc.vector.tensor_copy(o_sb, o_ps)
                    nc.sync.dma_start(
                        out=out.ap()[q0 * 128 : (q0 + 1) * 128, esl], in_=o_sb)
                tail_n[0] += 1

            def tail_seq(atn_src, qcc, qt_i, ecc, skip_last=False, pool=None):
                esl = slice(ecc * 512, (ecc + 1) * 512)
                o_ps = (pool or ps_at).tile(
                    [128, 512], F32, tag="ps" if pool is None else "sc")
                n_mm = pairs - 1 if skip_last else pairs
                for p in range(n_mm):
                    nc.tensor.matmul(
                        o_ps,
                        atn_src[p][:, qt_i * 128 : (qt_i + 1) * 128],
                        wo_sb[:, p, esl],
                        start=(p == 0),
                        stop=(not skip_last and p == pairs - 1),
                    )
                if skip_last:
                    return o_ps
                tail_evict(o_ps, qcc, qt_i, esl)
                return None

            def tail_finish(o_ps, atn_src, qcc, qt_i, ecc):
                esl = slice(ecc * 512, (ecc + 1) * 512)
                nc.tensor.matmul(
                    o_ps,
                    atn_src[pairs - 1][:, qt_i * 128 : (qt_i + 1) * 128],
                    wo_sb[:, pairs - 1, esl],
                    start=False,
                    stop=True,
                )
                tail_evict(o_ps, qcc, qt_i, esl)

            for sq in range(4 * ec_n - n_tail_defer, 4 * ec_n):
                tail_seq(prev_prev_atn, qc_n - 2, sq // ec_n, sq % ec_n)
            fill.drain()  # stragglers, all norm-independent by now
            # four partial qc3 sequences (pairs 0..2 only) keep the PE dense
            # while the final reciprocal chain runs on DVE; two borrow the
            # score-PSUM slots, which are free at this point
            heads = []
            for sq in range(4):
                o_ps = tail_seq(
                    prev_atn, qc_n - 1, sq // ec_n, sq % ec_n,
                    skip_last=True, pool=ps_sc if sq < 2 else None)
                heads.append((o_ps, sq))
            while pending_norm:
                norm_mms(pending_norm[0])
                norm_mul(pending_norm.pop(0))
            for o_ps, sq in heads:
                tail_finish(o_ps, prev_atn, qc_n - 1, sq // ec_n, sq % ec_n)
            for sq in range(4, 4 * ec_n):
                tail_seq(prev_atn, qc_n - 1, sq // ec_n, sq % ec_n)


    nc.compile()
    return nc


_PROGRAM_CACHE = {}


def _get_program(key):
    if key not in _PROGRAM_CACHE:
        _PROGRAM_CACHE[key] = build_program(*key)
    return _PROGRAM_CACHE[key]


def kernel(queries, keys, values, Wq, bq, Wk, bk, Wv, bv, Wo, bo):
    global LAST_EXEC_TIME_NS
    bf16 = ml_dtypes.bfloat16

    nc = _get_program((S, D, HC, D))

    xT = {}
    for name, arr in (("q", queries), ("k", keys), ("v", values)):
        xT[name] = [
            np.ascontiguousarray(np.asarray(arr[b]).T).astype(bf16)
            for b in range(B)
        ]
    Wq, Wk, Wv, Wo = (np.asarray(w) for w in (Wq, Wk, Wv, Wo))
    bqv, bkv, bvv = (np.asarray(v) for v in (bq, bk, bv))

    in_maps = []
    for c in range(N_CORES):
        b, g = c // 2, c % 2
        csl = slice(g * DPC, (g + 1) * DPC)
        in_maps.append(
            {
                "xqT": xT["q"][b],
                "xkT": xT["k"][b],
                "xvT": xT["v"][b],
                "wq": np.ascontiguousarray(Wq[:, csl]).astype(bf16),
                "wk": np.ascontiguousarray(Wk[:, csl]).astype(bf16),
                "wv": np.ascontiguousarray(Wv[:, csl]).astype(bf16),
                "wo": np.ascontiguousarray(Wo[csl, :]).astype(bf16),
                "bq": np.ascontiguousarray(bqv[csl]).astype(np.float32),
                "bk": np.ascontiguousarray(bkv[csl]).astype(np.float32),
                "bv": np.ascontiguousarray(bvv[csl]).astype(bf16),
            }
        )

    trace = os.environ.get("KERNEL_TRACE", "0") == "1"
    res = run_bass_kernel_spmd(nc, in_maps, list(range(N_CORES)), trace=trace)
    LAST_EXEC_TIME_NS = res.exec_time_ns

    bo = np.asarray(bo, dtype=np.float32)
    out = np.empty((B, S, D), dtype=np.float32)
    for b in range(B):
        out[b] = (
            np.asarray(res.results[2 * b]["out"], dtype=np.float32)
            + np.asarray(res.results[2 * b + 1]["out"], dtype=np.float32)
            + bo
        )
    return out


if __name__ == "__main__":
    rng = np.random.default_rng(0)
    t0 = time.time()
    nc = _get_program((S, D, HC, D))
    print(f"build+compile: {time.time() - t0:.1f}s")
